# revision 7
# baseline (speedup 1.0000x reference)
"""Bass/Trainium2 kernel for nn_KernelEdges (gnn_message_passing).

Computes A = exp((g_i + g_j - 2*Xf@Xf.T)/sigma^2) with zeroed diagonal,
broadcast to all B batch slots, where Xf = X.transpose(1,0,2).reshape(N, B*d).

Sharding: rows of the NxN pairwise matrix are split across 8 NeuronCores
(256 rows each).  The batch dim of the output is a pure replication of the
same [N, N] matrix, so each core writes only its unique [N/8, N] tile and
the host broadcasts to the B batch slots (as the reference itself does).

Each core receives a column-ROLLED copy of XT = Xf.T [B*d, N] so that its
own 256 columns sit at rolled positions 0..255; the matmul LHS (stationary
operand) is then a fixed slice of the streamed xt tile and no separate
lhst input is needed.  The host un-rolls the output columns after gather.

Math decomposition (exp(a+b) = exp(a)*exp(b)):
  psum[m, n] = sum_q xt_q[:, m_cols].T @ xt_q[:, n_block]   (Gram matrix)
  t = exp(-2/sigma^2 * psum + g_i/sigma^2)                  (ACT, bias/row)
  A = t * e_j,  e_j = exp(g_j/sigma^2)                      (DVE, row bcast)
The e_j row factor replaces the rank-1 g_j matmuls of earlier versions:
those burned ~5us of PE column-streaming; the DVE multiply rides an
otherwise idle engine, with the e_j row replicated across partitions once
by a gpsimd partition_broadcast.

Engine/queue budget (a DMA trigger costs ~600ns of issuing-engine time):
  tensor: 32 Gram matmuls only
  scalar: 8 ACTs only (the tail-driving stream; keep it unqueued)
  vector: bias trigger + 4 xt piece triggers, then 8 e_j multiplies
  gpsimd: ej trigger + 6 xt piece triggers + 2 partition_broadcasts
  sync:   6 xt piece triggers, then 8 output triggers
xt pieces stream nb-major so chain nb finishes (ACT+mul+store launch)
while pieces for nb+1 are still loading.

The diagonal is zeroed on the host (2K elements) after the gather.
"""

import numpy as np

B, N, D = 8, 2048, 64
NCORES = 8
R = N // NCORES          # 256 rows per core
KD = B * D               # 512 contraction dim
NB = 512                 # n-block (one PSUM bank of fp32)
NNB = N // NB            # 4 n-blocks
NMT = R // 128           # 2 m-tiles per core
NQ = KD // 128           # 4 k-tiles

MM_MODE = "bf16"         # matmul operand dtype ("bf16" | "f32r")
OUT_BF16 = True          # store A as bf16, upcast on host
EJ_MODE = "pb"           # "pb": e_j row via partition_broadcast + DVE mul
                         # "rank1": fold g_j into PSUM via rank-1 matmuls

# queue assignment for the 16 xt piece-load triggers (piece = nb*NQ+q).
# Only gpsimd/sync/scalar engines can issue DMAs; the assignment interleaves
# queues so arrival order roughly matches PE consumption order, with
# scalar's few triggers placed before its ACT stream needs to start
PIECE_QUEUE = {
    0: "sync", 1: "sync", 3: "sync", 4: "sync", 7: "sync", 10: "sync",
    12: "sync",
    2: "gpsimd", 6: "gpsimd", 9: "gpsimd", 13: "gpsimd", 14: "gpsimd",
    15: "gpsimd",
    5: "scalar", 8: "scalar", 11: "scalar",
}


def _build_program(inv_s2):
    import concourse.bass as bass
    import concourse.tile as tile
    from concourse import bacc, mybir

    f32 = mybir.dt.float32
    mm_dt = mybir.dt.bfloat16 if MM_MODE == "bf16" else mybir.dt.float32r
    out_dt = mybir.dt.bfloat16 if OUT_BF16 else f32

    nc = bacc.Bacc(
        "TRN2", target_bir_lowering=False, debug=False, num_devices=NCORES
    )

    GK = 2 if MM_MODE == "bf16" else 1  # g carried as hi+lo rows in bf16

    # xt pre-tiled on host: piece (nb, q) = rows (nb*NQ+q)*128..+128, fully
    # contiguous in DRAM for max DMA efficiency
    xt_d = nc.dram_tensor(
        "xt", [NNB * NQ * 128, NB], mm_dt, kind="ExternalInput"
    ).ap()
    bias_d = nc.dram_tensor("bias", [128, NMT], f32, kind="ExternalInput").ap()
    if EJ_MODE == "pb":
        ej_d = nc.dram_tensor("ej", [1, N], f32, kind="ExternalInput").ap()
    else:
        grow_d = nc.dram_tensor(
            "grow", [GK, N], mm_dt, kind="ExternalInput"
        ).ap()
    # out piece (mt, nb) at rows (mt*NNB+nb)*128..+128, contiguous
    out_d = nc.dram_tensor(
        "out", [NMT * NNB * 128, NB], out_dt, kind="ExternalOutput"
    ).ap()

    with tile.TileContext(nc) as tc:
        with (
            tc.tile_pool(name="persist", bufs=1) as persist,
            tc.tile_pool(name="apool", bufs=1) as apool,
            tc.tile_pool(name="psum", bufs=1, space="PSUM") as pspool,
        ):
            # ---- small loads ----
            bias_sb = persist.tile([128, NMT], f32, name="bias")
            nc.scalar.dma_start(bias_sb[:], bias_d[:])

            if EJ_MODE == "pb":
                ej_sb = persist.tile([1, N], f32, name="ej")
                nc.gpsimd.dma_start(ej_sb[:], ej_d[:])
                ejr_sb = persist.tile([128, N], f32, name="ejr")
            else:
                grow_sb = persist.tile([GK, N], mm_dt, name="grow")
                nc.gpsimd.dma_start(grow_sb[:], grow_d[:])
                neg_half = persist.tile([GK, 128], mm_dt, name="neg_half")
                if MM_MODE == "bf16":
                    nc.gpsimd.memset(
                        neg_half[:].bitcast(mybir.dt.uint16), 0xBF00
                    )
                else:
                    nc.gpsimd.memset(
                        neg_half[:].bitcast(mybir.dt.uint32), 0xBF000000
                    )

            # ---- xt piece loads, nb-major, spread across 3 DGE rings ----
            xt_sb = [
                persist.tile([128, N], mm_dt, name=f"xt{q}")
                for q in range(NQ)
            ]
            engines = {
                "sync": nc.sync, "scalar": nc.scalar, "gpsimd": nc.gpsimd
            }
            for idx in range(NNB * NQ):
                nb, q = divmod(idx, NQ)
                row0 = idx * 128
                engines[PIECE_QUEUE[idx]].dma_start(
                    xt_sb[q][:, nb * NB:(nb + 1) * NB],
                    xt_d[row0:row0 + 128, :],
                )
                if EJ_MODE == "pb" and idx in (2, 9):
                    # replicate e_j across partitions in halves, interleaved
                    # with gpsimd's triggers so each half is ready just
                    # before its first DVE multiply needs it
                    half = slice(0, N // 2) if idx == 2 else slice(N // 2, N)
                    nc.gpsimd.partition_broadcast(
                        ejr_sb[:, half], ej_sb[:, half]
                    )

            # ---- compute + store ----
            # all 8 accumulation chains live in the 8 PSUM banks at once
            ps = {
                (mt, nb): pspool.tile([128, NB], f32, name=f"ps{mt}{nb}")
                for nb in range(NNB) for mt in range(NMT)
            }
            if EJ_MODE == "rank1":
                for nb in range(NNB):
                    for mt in range(NMT):
                        nc.tensor.matmul(
                            ps[mt, nb][:],
                            neg_half[:],
                            grow_sb[:, nb * NB:(nb + 1) * NB],
                            start=True,
                            stop=False,
                        )
            a_tmp = {
                mt: apool.tile([128, N], f32, name=f"t{mt}")
                for mt in range(NMT)
            }
            a_sb = {
                mt: apool.tile([128, N], out_dt, name=f"a{mt}")
                for mt in range(NMT)
            }
            # matmul order matches piece arrival order (PE is in-order):
            # LHS is the core's own 256 rolled columns, a slice of piece
            # (nb=0, q) which is always already resident
            for nb in range(NNB):
                for q in range(NQ):
                    for mt in range(NMT):
                        nc.tensor.matmul(
                            ps[mt, nb][:],
                            xt_sb[q][:, mt * 128:(mt + 1) * 128],
                            xt_sb[q][:, nb * NB:(nb + 1) * NB],
                            start=(q == 0 and EJ_MODE == "pb"),
                            stop=(q == NQ - 1),
                        )
            # ACT + e_j multiply + store chase the chains in stop order
            for nb in range(NNB):
                for mt in range(NMT):
                    sl = slice(nb * NB, (nb + 1) * NB)
                    row0 = (mt * NNB + nb) * 128
                    if EJ_MODE == "pb":
                        nc.scalar.activation(
                            a_tmp[mt][:, sl],
                            ps[mt, nb][:],
                            mybir.ActivationFunctionType.Exp,
                            bias=bias_sb[:, mt:mt + 1],
                            scale=-2.0 * inv_s2,
                        )
                        nc.vector.tensor_mul(
                            a_sb[mt][:, sl], a_tmp[mt][:, sl], ejr_sb[:, sl]
                        )
                    else:
                        nc.scalar.activation(
                            a_sb[mt][:, sl],
                            ps[mt, nb][:],
                            mybir.ActivationFunctionType.Exp,
                            bias=bias_sb[:, mt:mt + 1],
                            scale=-2.0 * inv_s2,
                        )
                    nc.sync.dma_start(
                        out_d[row0:row0 + 128, :], a_sb[mt][:, sl]
                    )

    nc.compile()
    return nc


def _prepare(X, log_sigma):
    """Host prep: returns (inv_s2, in_maps) for run_bass_kernel_spmd."""
    import ml_dtypes

    X = np.ascontiguousarray(X, dtype=np.float32)
    assert X.shape == (B, N, D), X.shape

    sigma = float(np.exp(np.float32(log_sigma)))
    inv_s2 = 1.0 / (sigma * sigma)

    # XT[b*D+f, n] = X[b, n, f]
    XT = np.ascontiguousarray(X.transpose(0, 2, 1).reshape(KD, N))
    g = np.einsum("kn,kn->n", XT, XT).astype(np.float32)  # [N]

    mm_np = ml_dtypes.bfloat16 if MM_MODE == "bf16" else np.float32
    XTm = XT.astype(mm_np)

    in_maps = []
    for c in range(NCORES):
        r0 = c * R
        # roll columns so this core's own block is at rolled cols 0..R-1
        Xr = np.roll(XTm, -r0, axis=1)
        # pre-tile: piece (nb, q) contiguous -> [NNB*NQ*128, NB]
        xt_t = np.ascontiguousarray(
            Xr.reshape(NQ, 128, NNB, NB).transpose(2, 0, 1, 3)
        ).reshape(NNB * NQ * 128, NB)

        gr = np.roll(g, -r0)
        bias_np = np.empty((128, NMT), dtype=np.float32)
        for mt in range(NMT):
            bias_np[:, mt] = g[r0 + mt * 128: r0 + (mt + 1) * 128] * inv_s2
        im = {"xt": xt_t, "bias": bias_np}
        if EJ_MODE == "pb":
            im["ej"] = np.ascontiguousarray(
                np.exp(gr * inv_s2, dtype=np.float32)[None, :]
            )
        else:
            if MM_MODE == "bf16":
                g_hi = gr.astype(ml_dtypes.bfloat16)
                g_lo = (gr - g_hi.astype(np.float32)).astype(
                    ml_dtypes.bfloat16
                )
                im["grow"] = np.ascontiguousarray(np.stack([g_hi, g_lo]))
            else:
                im["grow"] = np.ascontiguousarray(gr[None, :])
        in_maps.append(im)
    return inv_s2, in_maps


def kernel(X, log_sigma):
    from concourse.bass_utils import run_bass_kernel_spmd

    inv_s2, in_maps = _prepare(X, log_sigma)
    nc = _build_program(inv_s2)
    res = run_bass_kernel_spmd(nc, in_maps, list(range(NCORES)))

    A = np.empty((N, N), dtype=np.float32)
    for c in range(NCORES):
        r0 = c * R
        t = np.asarray(res.results[c]["out"])
        # un-tile: [NMT*NNB*128, NB] -> [R, N] (still column-rolled)
        t = t.reshape(NMT, NNB, 128, NB).transpose(0, 2, 1, 3).reshape(R, N)
        # un-roll columns back to global positions
        A[r0:r0 + R, :] = np.roll(t.astype(np.float32), r0, axis=1)
    idx = np.arange(N)
    A[idx, idx] = 0.0
    out = np.empty((B, N, N), dtype=np.float32)
    out[:] = A[None, :, :]
    return out


# revision 8
# speedup vs baseline: 1.0703x; 1.0703x over previous
"""Bass/Trainium2 kernel for nn_KernelEdges (gnn_message_passing).

Computes A = exp((g_i + g_j - 2*Xf@Xf.T)/sigma^2) with zeroed diagonal,
broadcast to all B batch slots, where Xf = X.transpose(1,0,2).reshape(N, B*d).

Sharding: rows of the NxN pairwise matrix are split across 8 NeuronCores
(256 rows each).  The batch dim of the output is a pure replication of the
same [N, N] matrix, so each core writes only its unique [N/8, N] tile and
the host broadcasts to the B batch slots (as the reference itself does).

Each core receives a column-ROLLED copy of XT = Xf.T [B*d, N] so that its
own 256 columns sit at rolled positions 0..255; the matmul LHS (stationary
operand) is then a fixed slice of the streamed xt tile and no separate
lhst input is needed.  The host un-rolls the output columns after gather.

Math decomposition (exp(a+b) = exp(a)*exp(b)):
  psum[m, n] = sum_q xt_q[:, m_cols].T @ xt_q[:, n_block]   (Gram matrix)
  t = exp(-2/sigma^2 * psum + g_i/sigma^2)                  (ACT, bias/row)
  A = t * e_j,  e_j = exp(g_j/sigma^2)                      (DVE, row bcast)
The e_j row factor replaces rank-1 g_j matmuls (which cost ~5us of PE
column-streaming); e_j is replicated across partitions once by a gpsimd
partition_broadcast issued FIRST on that queue (ucode instructions stall
on the queue's outstanding DMAs, so nothing else may precede them).

DMA shape discipline: transfers are row-descriptor-rate limited (~6.5ns
per partition-row core-wide) until rows reach ~4KB, so every stream uses
>=2KB rows: xt streams as 8 half-tiles [128, 1024] bf16 (h-major, so the
h0 psum chains finish and their ACT+mul+store launch while h1 is still
loading), and the output leaves as 4 pieces [128, 1024] bf16.

Engine/queue budget (a DMA trigger costs ~600ns of issuing-engine time;
only gpsimd/sync/scalar can issue DMAs):
  tensor: 32 Gram matmuls only
  scalar: bias + 2 xt triggers, then 4 wide ACTs
  vector: 4 wide e_j multiplies (cannot DMA)
  gpsimd: ej trigger + partition_broadcast halves + 2 xt triggers
  sync:   4 xt triggers, then 4 output triggers

The diagonal is zeroed on the host (2K elements) after the gather.
"""

import numpy as np

B, N, D = 8, 2048, 64
NCORES = 8
R = N // NCORES          # 256 rows per core
KD = B * D               # 512 contraction dim
NB = 512                 # n-block (one PSUM bank of fp32)
NH = 2                   # column halves (streaming granularity)
HW = N // NH             # 1024 cols per half
NMT = R // 128           # 2 m-tiles per core
NQ = KD // 128           # 4 k-tiles

MM_MODE = "bf16"         # matmul operand dtype ("bf16" | "f32r")
OUT_BF16 = True          # store A as bf16, upcast on host
EJ_MODE = "pb"           # "pb": e_j row via partition_broadcast + DVE mul
                         # "rank1": fold g_j into PSUM via rank-1 matmuls

# queue for each xt half-tile trigger, piece idx = h*NQ+q
PIECE_QUEUE = {
    0: "sync", 1: "sync", 2: "sync", 3: "sync",
    4: "scalar", 5: "scalar",
    6: "gpsimd", 7: "gpsimd",
}


def _build_program(inv_s2):
    import concourse.bass as bass
    import concourse.tile as tile
    from concourse import bacc, mybir

    f32 = mybir.dt.float32
    mm_dt = mybir.dt.bfloat16 if MM_MODE == "bf16" else mybir.dt.float32r
    out_dt = mybir.dt.bfloat16 if OUT_BF16 else f32

    nc = bacc.Bacc(
        "TRN2", target_bir_lowering=False, debug=False, num_devices=NCORES
    )

    GK = 2 if MM_MODE == "bf16" else 1  # g carried as hi+lo rows in bf16

    # xt pre-tiled on host: piece (h, q) = rows (h*NQ+q)*128..+128, fully
    # contiguous in DRAM, 2KB rows
    xt_d = nc.dram_tensor(
        "xt", [NH * NQ * 128, HW], mm_dt, kind="ExternalInput"
    ).ap()
    bias_d = nc.dram_tensor("bias", [128, NMT], f32, kind="ExternalInput").ap()
    if EJ_MODE == "pb":
        ej_d = nc.dram_tensor("ej", [1, N], f32, kind="ExternalInput").ap()
    else:
        grow_d = nc.dram_tensor(
            "grow", [GK, N], mm_dt, kind="ExternalInput"
        ).ap()
    # out piece (mt, h) at rows (mt*NH+h)*128..+128, contiguous, 2KB rows
    out_d = nc.dram_tensor(
        "out", [NMT * NH * 128, HW], out_dt, kind="ExternalOutput"
    ).ap()

    with tile.TileContext(nc) as tc:
        with (
            tc.tile_pool(name="persist", bufs=1) as persist,
            tc.tile_pool(name="apool", bufs=1) as apool,
            tc.tile_pool(name="psum", bufs=1, space="PSUM") as pspool,
        ):
            # ---- gpsimd: ej load + partition broadcast FIRST ----
            if EJ_MODE == "pb":
                ej_sb = persist.tile([1, N], f32, name="ej")
                nc.gpsimd.dma_start(ej_sb[:], ej_d[:])
                ejr_sb = persist.tile([128, N], f32, name="ejr")
                for h in range(NH):
                    sl = slice(h * HW, (h + 1) * HW)
                    nc.gpsimd.partition_broadcast(ejr_sb[:, sl], ej_sb[:, sl])
            else:
                grow_sb = persist.tile([GK, N], mm_dt, name="grow")
                nc.gpsimd.dma_start(grow_sb[:], grow_d[:])
                neg_half = persist.tile([GK, 128], mm_dt, name="neg_half")
                if MM_MODE == "bf16":
                    nc.gpsimd.memset(
                        neg_half[:].bitcast(mybir.dt.uint16), 0xBF00
                    )
                else:
                    nc.gpsimd.memset(
                        neg_half[:].bitcast(mybir.dt.uint32), 0xBF000000
                    )

            bias_sb = persist.tile([128, NMT], f32, name="bias")
            nc.scalar.dma_start(bias_sb[:], bias_d[:])

            # ---- xt half-tile loads, h-major, spread across 3 DGE rings --
            xt_sb = [
                persist.tile([128, N], mm_dt, name=f"xt{q}")
                for q in range(NQ)
            ]
            engines = {
                "sync": nc.sync, "scalar": nc.scalar, "gpsimd": nc.gpsimd
            }
            for idx in range(NH * NQ):
                h, q = divmod(idx, NQ)
                row0 = idx * 128
                engines[PIECE_QUEUE[idx]].dma_start(
                    xt_sb[q][:, h * HW:(h + 1) * HW],
                    xt_d[row0:row0 + 128, :],
                )

            # ---- compute + store ----
            # 4 psum tiles of two banks each; chain (mt, h) spans both bank
            # halves so one wide ACT/mul/store covers it
            ps = {
                (mt, h): pspool.tile([128, HW], f32, name=f"ps{mt}{h}")
                for h in range(NH) for mt in range(NMT)
            }
            if EJ_MODE == "rank1":
                for h in range(NH):
                    for mt in range(NMT):
                        for nbh in range(2):
                            nc.tensor.matmul(
                                ps[mt, h][:, nbh * NB:(nbh + 1) * NB],
                                neg_half[:],
                                grow_sb[
                                    :, h * HW + nbh * NB:
                                    h * HW + (nbh + 1) * NB
                                ],
                                start=True,
                                stop=False,
                            )
            a_tmp = {
                mt: apool.tile([128, N], f32, name=f"t{mt}")
                for mt in range(NMT)
            }
            a_sb = {
                mt: apool.tile([128, N], out_dt, name=f"a{mt}")
                for mt in range(NMT)
            }
            # matmul order matches piece arrival order (PE is in-order);
            # within (h, q): mt-grouped so chain (mt0, h) stops before
            # (mt1, h) and the wide ACT pipeline starts earliest.
            # LHS is the core's own 256 rolled columns, a slice of piece
            # (h=0, q) which is always already resident
            for h in range(NH):
                for q in range(NQ):
                    for mt in range(NMT):
                        for nbh in range(2):
                            nc.tensor.matmul(
                                ps[mt, h][:, nbh * NB:(nbh + 1) * NB],
                                xt_sb[q][:, mt * 128:(mt + 1) * 128],
                                xt_sb[q][
                                    :, h * HW + nbh * NB:
                                    h * HW + (nbh + 1) * NB
                                ],
                                start=(q == 0 and EJ_MODE == "pb"),
                                stop=(q == NQ - 1),
                            )
            # wide ACT + e_j multiply + store chase the chains in stop order
            for h in range(NH):
                for mt in range(NMT):
                    sl = slice(h * HW, (h + 1) * HW)
                    row0 = (mt * NH + h) * 128
                    if EJ_MODE == "pb":
                        nc.scalar.activation(
                            a_tmp[mt][:, sl],
                            ps[mt, h][:],
                            mybir.ActivationFunctionType.Exp,
                            bias=bias_sb[:, mt:mt + 1],
                            scale=-2.0 * inv_s2,
                        )
                        nc.vector.tensor_mul(
                            a_sb[mt][:, sl], a_tmp[mt][:, sl], ejr_sb[:, sl]
                        )
                    else:
                        nc.scalar.activation(
                            a_sb[mt][:, sl],
                            ps[mt, h][:],
                            mybir.ActivationFunctionType.Exp,
                            bias=bias_sb[:, mt:mt + 1],
                            scale=-2.0 * inv_s2,
                        )
                    nc.sync.dma_start(
                        out_d[row0:row0 + 128, :], a_sb[mt][:, sl]
                    )

    nc.compile()
    return nc


def _prepare(X, log_sigma):
    """Host prep: returns (inv_s2, in_maps) for run_bass_kernel_spmd."""
    import ml_dtypes

    X = np.ascontiguousarray(X, dtype=np.float32)
    assert X.shape == (B, N, D), X.shape

    sigma = float(np.exp(np.float32(log_sigma)))
    inv_s2 = 1.0 / (sigma * sigma)

    # XT[b*D+f, n] = X[b, n, f]
    XT = np.ascontiguousarray(X.transpose(0, 2, 1).reshape(KD, N))
    g = np.einsum("kn,kn->n", XT, XT).astype(np.float32)  # [N]

    mm_np = ml_dtypes.bfloat16 if MM_MODE == "bf16" else np.float32
    XTm = XT.astype(mm_np)

    in_maps = []
    for c in range(NCORES):
        r0 = c * R
        # roll columns so this core's own block is at rolled cols 0..R-1
        Xr = np.roll(XTm, -r0, axis=1)
        # pre-tile: piece (h, q) contiguous -> [NH*NQ*128, HW]
        xt_t = np.ascontiguousarray(
            Xr.reshape(NQ, 128, NH, HW).transpose(2, 0, 1, 3)
        ).reshape(NH * NQ * 128, HW)

        gr = np.roll(g, -r0)
        bias_np = np.empty((128, NMT), dtype=np.float32)
        for mt in range(NMT):
            bias_np[:, mt] = g[r0 + mt * 128: r0 + (mt + 1) * 128] * inv_s2
        im = {"xt": xt_t, "bias": bias_np}
        if EJ_MODE == "pb":
            im["ej"] = np.ascontiguousarray(
                np.exp(gr * inv_s2, dtype=np.float32)[None, :]
            )
        else:
            if MM_MODE == "bf16":
                g_hi = gr.astype(ml_dtypes.bfloat16)
                g_lo = (gr - g_hi.astype(np.float32)).astype(
                    ml_dtypes.bfloat16
                )
                im["grow"] = np.ascontiguousarray(np.stack([g_hi, g_lo]))
            else:
                im["grow"] = np.ascontiguousarray(gr[None, :])
        in_maps.append(im)
    return inv_s2, in_maps


def kernel(X, log_sigma):
    from concourse.bass_utils import run_bass_kernel_spmd

    inv_s2, in_maps = _prepare(X, log_sigma)
    nc = _build_program(inv_s2)
    res = run_bass_kernel_spmd(nc, in_maps, list(range(NCORES)))

    A = np.empty((N, N), dtype=np.float32)
    for c in range(NCORES):
        r0 = c * R
        t = np.asarray(res.results[c]["out"])
        # un-tile: [NMT*NH*128, HW] -> [R, N] (still column-rolled)
        t = t.reshape(NMT, NH, 128, HW).transpose(0, 2, 1, 3).reshape(R, N)
        # un-roll columns back to global positions
        A[r0:r0 + R, :] = np.roll(t.astype(np.float32), r0, axis=1)
    idx = np.arange(N)
    A[idx, idx] = 0.0
    out = np.empty((B, N, N), dtype=np.float32)
    out[:] = A[None, :, :]
    return out


# revision 9
# speedup vs baseline: 1.0822x; 1.0111x over previous
"""Bass/Trainium2 kernel for nn_KernelEdges (gnn_message_passing).

Computes A = exp((g_i + g_j - 2*Xf@Xf.T)/sigma^2) with zeroed diagonal,
broadcast to all B batch slots, where Xf = X.transpose(1,0,2).reshape(N, B*d).

Sharding: rows of the NxN pairwise matrix are split across 8 NeuronCores
(256 rows each).  The batch dim of the output is a pure replication of the
same [N, N] matrix, so each core writes only its unique [N/8, N] tile and
the host broadcasts to the B batch slots (as the reference itself does).

Each core receives a column-ROLLED copy of XT = Xf.T [B*d, N] so that its
own 256 columns sit at rolled positions 0..255; the matmul LHS (stationary
operand) is then a fixed slice of the streamed xt tile and no separate
lhst input is needed.  The host un-rolls the output columns after gather.

Math decomposition (exp(a+b) = exp(a)*exp(b)):
  psum[m, n] = sum_q xt_q[:, m_cols].T @ xt_q[:, n_block]   (Gram matrix)
  t = exp(-2/sigma^2 * psum + g_i/sigma^2)                  (ACT, bias/row)
  A = t * e_j,  e_j = exp(g_j/sigma^2)                      (DVE, row bcast)
The e_j row factor replaces rank-1 g_j matmuls (which cost ~5us of PE
column-streaming); e_j is replicated across partitions once by a gpsimd
partition_broadcast issued FIRST on that queue (ucode instructions stall
on the queue's outstanding DMAs, so nothing else may precede them).

DMA shape discipline: transfers are row-descriptor-rate limited (~6.5ns
per partition-row core-wide) until rows reach ~4KB, so every stream uses
>=2KB rows: xt streams as 8 half-tiles [128, 1024] bf16 (h-major, so the
h0 psum chains finish and their ACT+mul+store launch while h1 is still
loading), and the output leaves as 4 pieces [128, 1024] bf16.

Engine/queue budget (a DMA trigger costs ~600ns of issuing-engine time;
only gpsimd/sync/scalar can issue DMAs):
  tensor: 32 Gram matmuls only
  scalar: bias + 2 xt triggers, then 4 wide ACTs
  vector: 4 wide e_j multiplies (cannot DMA)
  gpsimd: ej trigger + partition_broadcast halves + 2 xt triggers
  sync:   4 xt triggers, then 4 output triggers

The diagonal is zeroed on the host (2K elements) after the gather.
"""

import numpy as np

B, N, D = 8, 2048, 64
NCORES = 8
R = N // NCORES          # 256 rows per core
KD = B * D               # 512 contraction dim
NB = 512                 # n-block (one PSUM bank of fp32)
NH = 2                   # column halves (streaming granularity)
HW = N // NH             # 1024 cols per half
NMT = R // 128           # 2 m-tiles per core
NQ = KD // 128           # 4 k-tiles

MM_MODE = "bf16"         # matmul operand dtype ("bf16" | "f32r")
OUT_BF16 = True          # store A as bf16, upcast on host
EJ_MODE = "pb"           # "pb": e_j row via partition_broadcast + DVE mul
                         # "rank1": fold g_j into PSUM via rank-1 matmuls

# queue for each xt half-tile trigger, piece idx = h*NQ+q.
# ALL xt triggers ride one queue (sync): concurrent DMAs issued from
# different queues complete round-robin, which delays the FIRST piece (and
# the PE start) by several us; sequential triggers on one ring complete
# in order so the PE starts as soon as piece 0 lands.  gpsimd must carry
# no DMAs at all: its ucode partition_broadcast stalls until the engine's
# whole DMA queue drains (and the framework hoists triggers above ucode).
PIECE_QUEUE = {i: "sync" for i in range(8)}


def _build_program(inv_s2):
    import concourse.bass as bass
    import concourse.tile as tile
    from concourse import bacc, mybir

    f32 = mybir.dt.float32
    mm_dt = mybir.dt.bfloat16 if MM_MODE == "bf16" else mybir.dt.float32r
    out_dt = mybir.dt.bfloat16 if OUT_BF16 else f32

    nc = bacc.Bacc(
        "TRN2", target_bir_lowering=False, debug=False, num_devices=NCORES
    )

    GK = 2 if MM_MODE == "bf16" else 1  # g carried as hi+lo rows in bf16

    # xt pre-tiled on host: piece (h, q) = rows (h*NQ+q)*128..+128, fully
    # contiguous in DRAM, 2KB rows
    xt_d = nc.dram_tensor(
        "xt", [NH * NQ * 128, HW], mm_dt, kind="ExternalInput"
    ).ap()
    bias_d = nc.dram_tensor("bias", [128, NMT], f32, kind="ExternalInput").ap()
    if EJ_MODE == "pb":
        ej_d = nc.dram_tensor("ej", [1, N], f32, kind="ExternalInput").ap()
    else:
        grow_d = nc.dram_tensor(
            "grow", [GK, N], mm_dt, kind="ExternalInput"
        ).ap()
    # out piece (mt, h) at rows (mt*NH+h)*128..+128, contiguous, 2KB rows
    out_d = nc.dram_tensor(
        "out", [NMT * NH * 128, HW], out_dt, kind="ExternalOutput"
    ).ap()

    with tile.TileContext(nc) as tc:
        with (
            tc.tile_pool(name="persist", bufs=1) as persist,
            tc.tile_pool(name="apool", bufs=1) as apool,
            tc.tile_pool(name="psum", bufs=1, space="PSUM") as pspool,
        ):
            # ---- gpsimd: ej load + partition broadcast FIRST ----
            if EJ_MODE == "pb":
                ej_sb = persist.tile([1, N], f32, name="ej")
                nc.gpsimd.dma_start(ej_sb[:], ej_d[:])
                ejr_sb = persist.tile([128, N], f32, name="ejr")
                for h in range(NH):
                    sl = slice(h * HW, (h + 1) * HW)
                    nc.gpsimd.partition_broadcast(ejr_sb[:, sl], ej_sb[:, sl])
            else:
                grow_sb = persist.tile([GK, N], mm_dt, name="grow")
                nc.gpsimd.dma_start(grow_sb[:], grow_d[:])
                neg_half = persist.tile([GK, 128], mm_dt, name="neg_half")
                if MM_MODE == "bf16":
                    nc.gpsimd.memset(
                        neg_half[:].bitcast(mybir.dt.uint16), 0xBF00
                    )
                else:
                    nc.gpsimd.memset(
                        neg_half[:].bitcast(mybir.dt.uint32), 0xBF000000
                    )

            bias_sb = persist.tile([128, NMT], f32, name="bias")
            nc.scalar.dma_start(bias_sb[:], bias_d[:])

            # ---- xt half-tile loads, h-major, spread across 3 DGE rings --
            xt_sb = [
                persist.tile([128, N], mm_dt, name=f"xt{q}")
                for q in range(NQ)
            ]
            engines = {
                "sync": nc.sync, "scalar": nc.scalar, "gpsimd": nc.gpsimd
            }
            for idx in range(NH * NQ):
                h, q = divmod(idx, NQ)
                row0 = idx * 128
                engines[PIECE_QUEUE[idx]].dma_start(
                    xt_sb[q][:, h * HW:(h + 1) * HW],
                    xt_d[row0:row0 + 128, :],
                )

            # ---- compute + store ----
            # 4 psum tiles of two banks each; chain (mt, h) spans both bank
            # halves so one wide ACT/mul/store covers it
            ps = {
                (mt, h): pspool.tile([128, HW], f32, name=f"ps{mt}{h}")
                for h in range(NH) for mt in range(NMT)
            }
            if EJ_MODE == "rank1":
                for h in range(NH):
                    for mt in range(NMT):
                        for nbh in range(2):
                            nc.tensor.matmul(
                                ps[mt, h][:, nbh * NB:(nbh + 1) * NB],
                                neg_half[:],
                                grow_sb[
                                    :, h * HW + nbh * NB:
                                    h * HW + (nbh + 1) * NB
                                ],
                                start=True,
                                stop=False,
                            )
            a_tmp = {
                mt: apool.tile([128, N], f32, name=f"t{mt}")
                for mt in range(NMT)
            }
            a_sb = {
                mt: apool.tile([128, N], out_dt, name=f"a{mt}")
                for mt in range(NMT)
            }
            # matmul order matches piece arrival order (PE is in-order);
            # within (h, q): mt-grouped so chain (mt0, h) stops before
            # (mt1, h) and the wide ACT pipeline starts earliest.
            # LHS is the core's own 256 rolled columns, a slice of piece
            # (h=0, q) which is always already resident
            for h in range(NH):
                for q in range(NQ):
                    for mt in range(NMT):
                        for nbh in range(2):
                            nc.tensor.matmul(
                                ps[mt, h][:, nbh * NB:(nbh + 1) * NB],
                                xt_sb[q][:, mt * 128:(mt + 1) * 128],
                                xt_sb[q][
                                    :, h * HW + nbh * NB:
                                    h * HW + (nbh + 1) * NB
                                ],
                                start=(q == 0 and EJ_MODE == "pb"),
                                stop=(q == NQ - 1),
                            )
            # wide ACT + e_j multiply + store chase the chains in stop order
            for h in range(NH):
                for mt in range(NMT):
                    sl = slice(h * HW, (h + 1) * HW)
                    row0 = (mt * NH + h) * 128
                    if EJ_MODE == "pb":
                        nc.scalar.activation(
                            a_tmp[mt][:, sl],
                            ps[mt, h][:],
                            mybir.ActivationFunctionType.Exp,
                            bias=bias_sb[:, mt:mt + 1],
                            scale=-2.0 * inv_s2,
                        )
                        nc.vector.tensor_mul(
                            a_sb[mt][:, sl], a_tmp[mt][:, sl], ejr_sb[:, sl]
                        )
                    else:
                        nc.scalar.activation(
                            a_sb[mt][:, sl],
                            ps[mt, h][:],
                            mybir.ActivationFunctionType.Exp,
                            bias=bias_sb[:, mt:mt + 1],
                            scale=-2.0 * inv_s2,
                        )
                    nc.sync.dma_start(
                        out_d[row0:row0 + 128, :], a_sb[mt][:, sl]
                    )

    nc.compile()
    return nc


def _prepare(X, log_sigma):
    """Host prep: returns (inv_s2, in_maps) for run_bass_kernel_spmd."""
    import ml_dtypes

    X = np.ascontiguousarray(X, dtype=np.float32)
    assert X.shape == (B, N, D), X.shape

    sigma = float(np.exp(np.float32(log_sigma)))
    inv_s2 = 1.0 / (sigma * sigma)

    # XT[b*D+f, n] = X[b, n, f]
    XT = np.ascontiguousarray(X.transpose(0, 2, 1).reshape(KD, N))
    g = np.einsum("kn,kn->n", XT, XT).astype(np.float32)  # [N]

    mm_np = ml_dtypes.bfloat16 if MM_MODE == "bf16" else np.float32
    XTm = XT.astype(mm_np)

    in_maps = []
    for c in range(NCORES):
        r0 = c * R
        # roll columns so this core's own block is at rolled cols 0..R-1
        Xr = np.roll(XTm, -r0, axis=1)
        # pre-tile: piece (h, q) contiguous -> [NH*NQ*128, HW]
        xt_t = np.ascontiguousarray(
            Xr.reshape(NQ, 128, NH, HW).transpose(2, 0, 1, 3)
        ).reshape(NH * NQ * 128, HW)

        gr = np.roll(g, -r0)
        bias_np = np.empty((128, NMT), dtype=np.float32)
        for mt in range(NMT):
            bias_np[:, mt] = g[r0 + mt * 128: r0 + (mt + 1) * 128] * inv_s2
        im = {"xt": xt_t, "bias": bias_np}
        if EJ_MODE == "pb":
            im["ej"] = np.ascontiguousarray(
                np.exp(gr * inv_s2, dtype=np.float32)[None, :]
            )
        else:
            if MM_MODE == "bf16":
                g_hi = gr.astype(ml_dtypes.bfloat16)
                g_lo = (gr - g_hi.astype(np.float32)).astype(
                    ml_dtypes.bfloat16
                )
                im["grow"] = np.ascontiguousarray(np.stack([g_hi, g_lo]))
            else:
                im["grow"] = np.ascontiguousarray(gr[None, :])
        in_maps.append(im)
    return inv_s2, in_maps


def kernel(X, log_sigma):
    from concourse.bass_utils import run_bass_kernel_spmd

    inv_s2, in_maps = _prepare(X, log_sigma)
    nc = _build_program(inv_s2)
    res = run_bass_kernel_spmd(nc, in_maps, list(range(NCORES)))

    A = np.empty((N, N), dtype=np.float32)
    for c in range(NCORES):
        r0 = c * R
        t = np.asarray(res.results[c]["out"])
        # un-tile: [NMT*NH*128, HW] -> [R, N] (still column-rolled)
        t = t.reshape(NMT, NH, 128, HW).transpose(0, 2, 1, 3).reshape(R, N)
        # un-roll columns back to global positions
        A[r0:r0 + R, :] = np.roll(t.astype(np.float32), r0, axis=1)
    idx = np.arange(N)
    A[idx, idx] = 0.0
    out = np.empty((B, N, N), dtype=np.float32)
    out[:] = A[None, :, :]
    return out


# revision 11
# speedup vs baseline: 1.1555x; 1.0677x over previous
"""Bass/Trainium2 kernel for nn_KernelEdges (gnn_message_passing).

Computes A = exp((g_i + g_j - 2*Xf@Xf.T)/sigma^2) with zeroed diagonal,
broadcast to all B batch slots, where Xf = X.transpose(1,0,2).reshape(N, B*d).

Sharding: rows of the NxN pairwise matrix are split across 8 NeuronCores
(256 rows each).  The batch dim of the output is a pure replication of the
same [N, N] matrix, so each core writes only its unique [N/8, N] tile and
the host broadcasts to the B batch slots (as the reference itself does).

Each core receives a column-ROLLED copy of XT = Xf.T [B*d, N] so that its
own 256 columns sit at rolled positions 0..255; the matmul LHS (stationary
operand) is then a fixed slice of the streamed xt tile and no separate
lhst input is needed.  The host un-rolls the output columns after gather.

Math decomposition (exp(a+b) = exp(a)*exp(b)):
  psum[m, n] = sum_q xt_q[:, m_cols].T @ xt_q[:, n_block]   (Gram matrix)
  t = exp(-2/sigma^2 * psum + g_i/sigma^2)                  (ACT, bias/row)
  A = t * e_j,  e_j = exp(g_j/sigma^2)                      (DVE, row bcast)
The e_j row factor replaces rank-1 g_j matmuls (which cost ~5us of PE
column-streaming); e_j is replicated across partitions once by a gpsimd
partition_broadcast issued FIRST on that queue (ucode instructions stall
on the queue's outstanding DMAs, so nothing else may precede them).

DMA shape discipline: transfers are row-descriptor-rate limited (~6.5ns
per partition-row core-wide) until rows reach ~4KB, so every stream uses
>=2KB rows: xt streams as 8 half-tiles [128, 1024] bf16 (h-major, so the
h0 psum chains finish and their ACT+mul+store launch while h1 is still
loading), and the output leaves as 4 pieces [128, 1024] bf16.

Engine/queue budget (a DMA trigger costs ~600ns of issuing-engine time;
only gpsimd/sync/scalar can issue DMAs):
  tensor: 32 Gram matmuls only
  scalar: bias + 2 xt triggers, then 4 wide ACTs
  vector: 4 wide e_j multiplies (cannot DMA)
  gpsimd: ej trigger + partition_broadcast halves + 2 xt triggers
  sync:   4 xt triggers, then 4 output triggers

The diagonal is zeroed on the host (2K elements) after the gather.
"""

import numpy as np

B, N, D = 8, 2048, 64
NCORES = 8
R = N // NCORES          # 256 rows per core
KD = B * D               # 512 contraction dim
NB = 512                 # n-block (one PSUM bank of fp32)
NH = 2                   # column halves (streaming granularity)
HW = N // NH             # 1024 cols per half
NMT = R // 128           # 2 m-tiles per core
NQ = KD // 128           # 4 k-tiles

MM_MODE = "bf16"         # matmul operand dtype ("bf16" | "f32r")
OUT_BF16 = True          # store A as bf16, upcast on host
# "rank1": fold g_j into PSUM via rank-1 matmuls (~8 extra PE matmuls).
# "pb" (e_j row via gpsimd partition_broadcast + DVE multiply) is kept for
# reference but measured WORSE: the gpsimd ucode instruction behaves as a
# barrier against all prior work, running only after the last matmul and
# serializing a ~13us tail of multiplies + stores.
EJ_MODE = "rank1"

# queue for each xt half-tile trigger, piece idx = h*NQ+q.
# ALL xt triggers ride one queue (sync): concurrent DMAs issued from
# different queues complete round-robin, which delays the FIRST piece (and
# the PE start) by several us; sequential triggers on one ring complete
# in order so the PE starts as soon as piece 0 lands.  gpsimd must carry
# no DMAs at all: its ucode partition_broadcast stalls until the engine's
# whole DMA queue drains (and the framework hoists triggers above ucode).
PIECE_QUEUE = {i: "sync" for i in range(8)}


def _build_program(inv_s2):
    import concourse.bass as bass
    import concourse.tile as tile
    from concourse import bacc, mybir

    f32 = mybir.dt.float32
    mm_dt = mybir.dt.bfloat16 if MM_MODE == "bf16" else mybir.dt.float32r
    out_dt = mybir.dt.bfloat16 if OUT_BF16 else f32

    nc = bacc.Bacc(
        "TRN2", target_bir_lowering=False, debug=False, num_devices=NCORES
    )

    GK = 2 if MM_MODE == "bf16" else 1  # g carried as hi+lo rows in bf16

    # xt pre-tiled on host: piece (h, q) = rows (h*NQ+q)*128..+128, fully
    # contiguous in DRAM, 2KB rows
    xt_d = nc.dram_tensor(
        "xt", [NH * NQ * 128, HW], mm_dt, kind="ExternalInput"
    ).ap()
    bias_d = nc.dram_tensor("bias", [128, NMT], f32, kind="ExternalInput").ap()
    if EJ_MODE == "pb":
        ej_d = nc.dram_tensor("ej", [1, N], f32, kind="ExternalInput").ap()
    else:
        grow_d = nc.dram_tensor(
            "grow", [GK, N], mm_dt, kind="ExternalInput"
        ).ap()
    # out piece (mt, h) at rows (mt*NH+h)*128..+128, contiguous, 2KB rows
    out_d = nc.dram_tensor(
        "out", [NMT * NH * 128, HW], out_dt, kind="ExternalOutput"
    ).ap()

    with tile.TileContext(nc) as tc:
        with (
            tc.tile_pool(name="persist", bufs=1) as persist,
            tc.tile_pool(name="apool", bufs=1) as apool,
            tc.tile_pool(name="psum", bufs=1, space="PSUM") as pspool,
        ):
            # ---- gpsimd: ej load + partition broadcast FIRST ----
            if EJ_MODE == "pb":
                ej_sb = persist.tile([1, N], f32, name="ej")
                nc.gpsimd.dma_start(ej_sb[:], ej_d[:])
                ejr_sb = persist.tile([128, N], f32, name="ejr")
                for h in range(NH):
                    sl = slice(h * HW, (h + 1) * HW)
                    nc.gpsimd.partition_broadcast(ejr_sb[:, sl], ej_sb[:, sl])
            else:
                grow_sb = persist.tile([GK, N], mm_dt, name="grow")
                nc.gpsimd.dma_start(grow_sb[:], grow_d[:])
                neg_half = persist.tile([GK, 128], mm_dt, name="neg_half")
                if MM_MODE == "bf16":
                    nc.gpsimd.memset(
                        neg_half[:].bitcast(mybir.dt.uint16), 0xBF00
                    )
                else:
                    nc.gpsimd.memset(
                        neg_half[:].bitcast(mybir.dt.uint32), 0xBF000000
                    )

            bias_sb = persist.tile([128, NMT], f32, name="bias")
            nc.scalar.dma_start(bias_sb[:], bias_d[:])

            # ---- xt half-tile loads, h-major, spread across 3 DGE rings --
            xt_sb = [
                persist.tile([128, N], mm_dt, name=f"xt{q}")
                for q in range(NQ)
            ]
            engines = {
                "sync": nc.sync, "scalar": nc.scalar, "gpsimd": nc.gpsimd
            }
            for idx in range(NH * NQ):
                h, q = divmod(idx, NQ)
                row0 = idx * 128
                engines[PIECE_QUEUE[idx]].dma_start(
                    xt_sb[q][:, h * HW:(h + 1) * HW],
                    xt_d[row0:row0 + 128, :],
                )

            # ---- compute + store ----
            # 4 psum tiles of two banks each; chain (mt, h) spans both bank
            # halves so one wide ACT/mul/store covers it
            ps = {
                (mt, h): pspool.tile([128, HW], f32, name=f"ps{mt}{h}")
                for h in range(NH) for mt in range(NMT)
            }
            if EJ_MODE == "pb":
                a_tmp = {
                    mt: apool.tile([128, N], f32, name=f"t{mt}")
                    for mt in range(NMT)
                }
            a_sb = {
                mt: apool.tile([128, N], out_dt, name=f"a{mt}")
                for mt in range(NMT)
            }
            # matmul order matches piece arrival order (PE is in-order);
            # within (h, q): mt-grouped so chain (mt0, h) stops before
            # (mt1, h) and the wide ACT pipeline starts earliest.
            # LHS is the core's own 256 rolled columns, a slice of piece
            # (h=0, q) which is always already resident.
            # rank-1 seeds interleave per-half: they depend only on grow
            # (arrives first), so h0's seeds warm the PE before piece 0
            # lands and h1's seeds hide in the stream
            for h in range(NH):
                if EJ_MODE == "rank1":
                    for mt in range(NMT):
                        for nbh in range(2):
                            nc.tensor.matmul(
                                ps[mt, h][:, nbh * NB:(nbh + 1) * NB],
                                neg_half[:],
                                grow_sb[
                                    :, h * HW + nbh * NB:
                                    h * HW + (nbh + 1) * NB
                                ],
                                start=True,
                                stop=False,
                            )
                for q in range(NQ):
                    for mt in range(NMT):
                        for nbh in range(2):
                            nc.tensor.matmul(
                                ps[mt, h][:, nbh * NB:(nbh + 1) * NB],
                                xt_sb[q][:, mt * 128:(mt + 1) * 128],
                                xt_sb[q][
                                    :, h * HW + nbh * NB:
                                    h * HW + (nbh + 1) * NB
                                ],
                                start=(q == 0 and EJ_MODE == "pb"),
                                stop=(q == NQ - 1),
                            )
            # wide ACT + e_j multiply + store chase the chains in stop order
            for h in range(NH):
                for mt in range(NMT):
                    sl = slice(h * HW, (h + 1) * HW)
                    row0 = (mt * NH + h) * 128
                    if EJ_MODE == "pb":
                        nc.scalar.activation(
                            a_tmp[mt][:, sl],
                            ps[mt, h][:],
                            mybir.ActivationFunctionType.Exp,
                            bias=bias_sb[:, mt:mt + 1],
                            scale=-2.0 * inv_s2,
                        )
                        nc.vector.tensor_mul(
                            a_sb[mt][:, sl], a_tmp[mt][:, sl], ejr_sb[:, sl]
                        )
                    else:
                        nc.scalar.activation(
                            a_sb[mt][:, sl],
                            ps[mt, h][:],
                            mybir.ActivationFunctionType.Exp,
                            bias=bias_sb[:, mt:mt + 1],
                            scale=-2.0 * inv_s2,
                        )
                    nc.sync.dma_start(
                        out_d[row0:row0 + 128, :], a_sb[mt][:, sl]
                    )

    nc.compile()
    return nc


def _prepare(X, log_sigma):
    """Host prep: returns (inv_s2, in_maps) for run_bass_kernel_spmd."""
    import ml_dtypes

    X = np.ascontiguousarray(X, dtype=np.float32)
    assert X.shape == (B, N, D), X.shape

    sigma = float(np.exp(np.float32(log_sigma)))
    inv_s2 = 1.0 / (sigma * sigma)

    # XT[b*D+f, n] = X[b, n, f]
    XT = np.ascontiguousarray(X.transpose(0, 2, 1).reshape(KD, N))
    g = np.einsum("kn,kn->n", XT, XT).astype(np.float32)  # [N]

    mm_np = ml_dtypes.bfloat16 if MM_MODE == "bf16" else np.float32
    XTm = XT.astype(mm_np)

    in_maps = []
    for c in range(NCORES):
        r0 = c * R
        # roll columns so this core's own block is at rolled cols 0..R-1
        Xr = np.roll(XTm, -r0, axis=1)
        # pre-tile: piece (h, q) contiguous -> [NH*NQ*128, HW]
        xt_t = np.ascontiguousarray(
            Xr.reshape(NQ, 128, NH, HW).transpose(2, 0, 1, 3)
        ).reshape(NH * NQ * 128, HW)

        gr = np.roll(g, -r0)
        bias_np = np.empty((128, NMT), dtype=np.float32)
        for mt in range(NMT):
            bias_np[:, mt] = g[r0 + mt * 128: r0 + (mt + 1) * 128] * inv_s2
        im = {"xt": xt_t, "bias": bias_np}
        if EJ_MODE == "pb":
            im["ej"] = np.ascontiguousarray(
                np.exp(gr * inv_s2, dtype=np.float32)[None, :]
            )
        else:
            if MM_MODE == "bf16":
                g_hi = gr.astype(ml_dtypes.bfloat16)
                g_lo = (gr - g_hi.astype(np.float32)).astype(
                    ml_dtypes.bfloat16
                )
                im["grow"] = np.ascontiguousarray(np.stack([g_hi, g_lo]))
            else:
                im["grow"] = np.ascontiguousarray(gr[None, :])
        in_maps.append(im)
    return inv_s2, in_maps


def kernel(X, log_sigma):
    from concourse.bass_utils import run_bass_kernel_spmd

    inv_s2, in_maps = _prepare(X, log_sigma)
    nc = _build_program(inv_s2)
    res = run_bass_kernel_spmd(nc, in_maps, list(range(NCORES)))

    A = np.empty((N, N), dtype=np.float32)
    for c in range(NCORES):
        r0 = c * R
        t = np.asarray(res.results[c]["out"])
        # un-tile: [NMT*NH*128, HW] -> [R, N] (still column-rolled)
        t = t.reshape(NMT, NH, 128, HW).transpose(0, 2, 1, 3).reshape(R, N)
        # un-roll columns back to global positions
        A[r0:r0 + R, :] = np.roll(t.astype(np.float32), r0, axis=1)
    idx = np.arange(N)
    A[idx, idx] = 0.0
    out = np.empty((B, N, N), dtype=np.float32)
    out[:] = A[None, :, :]
    return out
